# revision 25
# baseline (speedup 1.0000x reference)
"""Trainium2 Bass kernel for BottleNeck attention (8 NeuronCores).

Reference computation (jax, fp32):
    qp = q @ Wq.T + bq          [B=8, L=4096, D=1024]
    kp = k @ Wk.T + bk
    vp = v @ Wv.T + bv
    score = qp[:, :256] @ kp.T / sqrt(D)        [B, 256, L]
    attn  = softmax(score, axis=0)              (softmax over the BATCH axis!)
    out   = attn @ vp                           [B, 256, D]

Strategy:
  * Data-parallel over batch: core c owns batch b=c.
  * Algebraic reassociation avoids projecting full-length k/v:
        qp_T = Wq.T.T @ q.T                     [D, Q]     (per batch)
        qk_T = Wk.T @ qp_T                      [D, Q]
        score_T = k.T.T @ qk_T (+ bias row)     [L, Q]
        E = exp(score_T / 32)
        denom = AllReduce_batch(E)              (axis-0 softmax denominator)
        attn_T = E / denom                      [L, Q]
        av_T = v.T-chunks @ attn_T              [D, Q]
        out = av_T.T @ Wv.T + rowsum(attn) * bv [Q, D]
    This cuts FLOPs ~3x vs projecting kp/vp at full length.
  * Host pre-transposes (k.T, q.T, Wq.T, Wv.T) so no transposes on device.
  * Compute dtype bf16 (fp32 PSUM accumulation); the AllReduce payload is
    fp16 (exp scores are positive and bounded, so fp16 rounding ~2.4e-4).

Scheduling (measured on this fleet):
  * ncfw/TOPSP collectives cannot BEGIN their mesh until the firmware is
    warm: the CC processes the first trigger somewhere in ~60-85us after
    NEFF start (run-to-run jitter, uncorrelated with trigger time), the
    mesh begins exactly 11.1us after that, and its first sync completes
    when the SLOWEST core's firmware is in (~84-113us observed). All local
    compute before that wall is free, so phases A/B/C (~60us: projections,
    scores, exps) fill the pre-wall window with the contraction index
    outermost and PSUM-resident chains.
  * The mesh has a ~17.5us FIXED cost per collective chunk (a 0.13MB
    chunk's data events take 16.6us vs 24.5us for 1.31MB, ~16us/MB
    marginal), so few big chunks win: E-AllReduce in 2 chunks [20, 12]
    l-chunks, each stored + triggered as soon as phase C finishes it (all
    triggers land well before the wall, so the CC drains them
    back-to-back). An fp8e4 wire was tried (exec 182us) but the mesh
    accumulates in the wire dtype and rel err hit 2.6e-2 (> 2e-2 gate).
  * Phase F critical path after a chunk's denominator lands: dn load +
    fp16->f32 cast on the Scalar queue -> reciprocal_approx_fast + E*recip
    mult on Vector -> av matmuls (~3.5-4.4us lead-in). The dn loads/casts
    are emitted INSIDE phase C, right after their own chunk's collective
    and BEFORE the next chunk's: Tile's semaphore accounting otherwise
    makes the attn chain wait for the next chunk's trigger instruction,
    which only executes ~3us after the prior mesh completes (this false
    dependency cost the old layout ~8-20us of lead-in).
  * rowsum(attn) is accumulated on Vector (bf16 adds per l-chunk) and
    reduced across partitions with a single 0.24us PE matmul at the end --
    this removes 32 rider matmuls (~7us) from the PE's critical path and
    lets av use 8 clean PSUM banks.
  * kT fully prefetched up front; v (8MB) + Wv DMAs deferred until the
    first chunk's E store so the AllReduce windows run on quiet HBM.
  * Post-denom0 critical path is now ~58us: lead-in 4.4 + av 37 (256
    matmuls at ~145ns, PE-bound, <7us of bubbles) + rowsum/copies 2 +
    out-proj 10 + drain 2.4. Exec = denom0 + ~58us; denom0 = wall + 26.5.
"""

import sys
from contextlib import ExitStack

sys.path.insert(0, "/opt/trn_rl_repo")

import numpy as np

import concourse.bass as bass
import concourse.mybir as mybir
import concourse.tile as tile
from concourse import bacc, bass_utils

B = 8
L = 4096
D = 1024
Q = 256  # bottleneck
N_CORES = 8
P = 128
DC = D // P  # 8 d-chunks
EC = D // P  # 8 e-chunks
LC = L // P  # 32 l-chunks
SCALE = 1.0 / 32.0  # 1/sqrt(1024)

# compute dtype for matmul operands ("bf16" | "fp32r" | "fp32")
COMPUTE = "bf16"

# AllReduce chunk sizes in l-chunks. The mesh has a ~17.5us FIXED cost per
# chunk (measured: a 0.13MB chunk's data events take 16.6us vs 24.5us for
# 1.31MB), so few big chunks win; [20,12] balances the first denominator's
# arrival against the exposed last-chunk tail.
AR_LCS = [20, 12]

_CDT = {
    "bf16": mybir.dt.bfloat16,
    "fp32r": mybir.dt.float32r,  # fp32 bits; PE rounds internally (~tf32)
    "fp32": mybir.dt.float32,
}

_cached = {}


def _np_cdt():
    if COMPUTE == "bf16":
        import ml_dtypes

        return np.dtype(ml_dtypes.bfloat16)
    return np.dtype(np.float32)


def build_kernel():
    CDT = _CDT[COMPUTE]
    F32 = mybir.dt.float32

    nc = bacc.Bacc("TRN2", target_bir_lowering=False, debug=False,
                   num_devices=N_CORES)

    # ---- per-core external inputs (host pre-transposed / pre-cast) ----
    kT = nc.dram_tensor("kT", [D, L], CDT, kind="ExternalInput")       # k[b].T
    v_in = nc.dram_tensor("v_in", [L, D], CDT, kind="ExternalInput")   # v[b]
    qT = nc.dram_tensor("qT", [D, Q], CDT, kind="ExternalInput")       # q[b,:Q].T
    wqT = nc.dram_tensor("wqT", [D, D], CDT, kind="ExternalInput")     # Wq.T
    wk = nc.dram_tensor("wk", [D, D], CDT, kind="ExternalInput")       # Wk
    wvT = nc.dram_tensor("wvT", [D, D], CDT, kind="ExternalInput")     # Wv.T
    bq_in = nc.dram_tensor("bq_in", [1, D], CDT, kind="ExternalInput")
    bk_in = nc.dram_tensor("bk_in", [P, EC], CDT, kind="ExternalInput")  # bk.reshape(EC,P).T
    bv_in = nc.dram_tensor("bv_in", [1, D], CDT, kind="ExternalInput")
    ones_r_in = nc.dram_tensor("ones_r", [1, Q], CDT, kind="ExternalInput")
    ones_c_in = nc.dram_tensor("ones_c", [P, 1], CDT, kind="ExternalInput")
    out_ext = nc.dram_tensor("out", [Q, D], CDT, kind="ExternalOutput")

    # DRAM views with the partition-chunk structure we DMA through
    kT_v = kT.rearrange("(c p) l -> p c l", p=P)        # [128, 8, 4096]
    wqT_v = wqT.rearrange("(c p) e -> p c e", p=P)      # [128, 8, 1024]
    wk_v = wk.rearrange("(c p) d -> p c d", p=P)
    wvT_v = wvT.rearrange("(c p) e -> p c e", p=P)
    qT_v = qT.rearrange("(c p) q -> p c q", p=P)        # [128, 8, 256]
    out_v = out_ext.rearrange("(m p) e -> p m e", p=P)  # [128, 2, 1024]

    with tile.TileContext(nc) as tc, ExitStack() as top:
        consts = top.enter_context(tc.tile_pool(name="consts", bufs=1))
        qstate = top.enter_context(tc.tile_pool(name="qstate", bufs=1))
        dram = top.enter_context(tc.tile_pool(name="dram", bufs=1, space="DRAM"))

        # ---------------- constants ----------------
        ones_row = consts.tile([1, Q], CDT)       # [1, 256] of 1.0
        ones_col = consts.tile([P, 1], CDT)       # [128, 1] of 1.0
        nc.sync.dma_start(out=ones_row, in_=ones_r_in[:, :])
        nc.sync.dma_start(out=ones_col, in_=ones_c_in[:, :])
        bq_sb = consts.tile([1, D], CDT)
        bk_sb = consts.tile([P, EC], CDT)
        bv_sb = consts.tile([1, D], CDT)
        nc.sync.dma_start(out=bq_sb, in_=bq_in[:, :])
        nc.sync.dma_start(out=bk_sb, in_=bk_in[:, :])
        nc.sync.dma_start(out=bv_sb, in_=bv_in[:, :])

        ART = mybir.dt.float16  # AllReduce payload dtype (E fits fp16 range)

        # Scalar-queue warmup: a dummy activation with no upstream compute
        # deps loads the EXP table and wakes the Scalar queue at ~7us, so
        # the first real EXP fires at data-readiness instead of ~50us.
        warm = consts.tile([1, Q], ART)
        nc.scalar.activation(out=warm, in_=ones_row,
                             func=mybir.ActivationFunctionType.Exp,
                             scale=SCALE)

        qpT_sb = qstate.tile([P, EC, Q], CDT)
        qkT_sb = qstate.tile([P, DC, Q], CDT)
        qkb_sb = qstate.tile([1, Q], CDT)
        avT_sb = qstate.tile([P, DC, Q], CDT)
        rs_acc = qstate.tile([P, Q], CDT)   # rowsum(attn) partial, per part.
        rs_sb = qstate.tile([1, Q], CDT)

        SLAB = 4  # l-chunks per kT slab (512 l positions)
        kslab_ctx = ExitStack()
        kslabs = kslab_ctx.enter_context(tc.tile_pool(name="kslabs", bufs=6))

        # ================ phases A+B: q-side projections ================
        # Contraction index outermost with all PSUM chains resident: the
        # first matmuls need only the first 128-row chunk of the weight.
        # Each phase is split in two halves to bound live PSUM.
        with tc.tile_pool(name="wab", bufs=1) as wab, \
             tc.tile_pool(name="psAB", bufs=4, space="PSUM") as psAB, \
             tc.tile_pool(name="psbias", bufs=1, space="PSUM") as psbias:
            wqT_sb = wab.tile([P, EC, D], CDT)
            wk_sb = wab.tile([P, EC, D], CDT)
            qT_sb = wab.tile([P, DC, Q], CDT)
            # DMA order = consumption order: per-dc (wqT,qT) for A, then
            # the first kT slab, then per-ec wk for B, second kT slab.
            for dc in range(DC):
                nc.sync.dma_start(out=wqT_sb[:, dc, :], in_=wqT_v[:, dc, :])
                nc.sync.dma_start(out=qT_sb[:, dc, :], in_=qT_v[:, dc, :])
            kT_pre = []
            for sl in range(2):
                kT_t = kslabs.tile([P, DC, SLAB * P], CDT, tag="kT",
                                   name=f"kT_pre{sl}")
                nc.sync.dma_start(
                    out=kT_t, in_=kT_v[:, :, sl * SLAB * P:(sl + 1) * SLAB * P])
                kT_pre.append(kT_t)
            for ec in range(EC):
                nc.sync.dma_start(out=wk_sb[:, ec, :], in_=wk_v[:, ec, :])
            # prefetch the REMAINING kT slabs now (8 bufs, no rotation):
            # their DMAs must issue before the deferred v prefetch so the
            # later AllReduce chunks' scores are never DMA-starved.
            for sl in range(2, LC // SLAB):
                kT_t = kslabs.tile([P, DC, SLAB * P], CDT, tag="kT",
                                   name=f"kT_pre{sl}")
                nc.sync.dma_start(
                    out=kT_t, in_=kT_v[:, :, sl * SLAB * P:(sl + 1) * SLAB * P])
                kT_pre.append(kT_t)

            # phase A: qp_T[e,q] = sum_d WqT[d, e-chunk].T @ qT[d, q] + bq
            for half in range(2):
                ecs = range(half * 4, half * 4 + 4)
                psA = [psAB.tile([P, Q], F32, tag="ab", name=f"psA_{half}_{i}")
                       for i in range(4)]
                for dc in range(DC):
                    for i, ec in enumerate(ecs):
                        nc.tensor.matmul(
                            psA[i],
                            wqT_sb[:, dc, ec * P:(ec + 1) * P],
                            qT_sb[:, dc, :],
                            start=(dc == 0), stop=False,
                        )
                for i, ec in enumerate(ecs):
                    nc.tensor.matmul(
                        psA[i], bq_sb[:, ec * P:(ec + 1) * P], ones_row,
                        start=False, stop=True,
                    )
                    nc.vector.tensor_copy(qpT_sb[:, ec, :], psA[i])

            # phase B: qk_T[d,q] = sum_e Wk[e, d-chunk].T @ qp_T[e, q]
            ps_qkb = psbias.tile([1, Q], F32, name="ps_qkb")
            for half in range(2):
                dcs = range(half * 4, half * 4 + 4)
                psB = [psAB.tile([P, Q], F32, tag="ab", name=f"psB_{half}_{i}")
                       for i in range(4)]
                for ec in range(EC):
                    for i, dc in enumerate(dcs):
                        nc.tensor.matmul(
                            psB[i],
                            wk_sb[:, ec, dc * P:(dc + 1) * P],
                            qpT_sb[:, ec, :],
                            start=(ec == 0), stop=(ec == EC - 1),
                        )
                for i, dc in enumerate(dcs):
                    nc.vector.tensor_copy(qkT_sb[:, dc, :], psB[i])
            # score bias row: qkb[q] = sum_e bk[e] * qp_T[e, q]
            for ec in range(EC):
                nc.tensor.matmul(
                    ps_qkb, bk_sb[:, ec:ec + 1], qpT_sb[:, ec, :],
                    start=(ec == 0), stop=(ec == EC - 1),
                )
            nc.vector.tensor_copy(qkb_sb, ps_qkb)

        # ================ phase C: score_T -> E -> DRAM ================
        # NOTE: an fp8e4 AllReduce wire was tried (halves mesh bytes, exec
        # 182us) but the mesh accumulates in the wire dtype and the compound
        # rounding pushed rel err to 2.6e-2 (> the 2e-2 gate). fp16 it is.
        bigctx = ExitStack()
        bigbuf = bigctx.enter_context(tc.tile_pool(name="bigbuf", bufs=1))
        wvp = bigctx.enter_context(tc.tile_pool(name="wvp", bufs=1))
        E_sb = bigbuf.tile([P, LC * Q], ART)          # [128, 8192]
        wvT_sb = wvp.tile([P, DC, D], CDT)
        v_all = bigbuf.tile([P, LC, D], CDT)
        v_pv = v_in.rearrange("(c p) d -> p c d", p=P)  # [128, 32, 1024]
        assert sum(AR_LCS) == LC
        ar_starts = [sum(AR_LCS[:i]) for i in range(len(AR_LCS))]
        ar_ends = [ar_starts[i] + AR_LCS[i] for i in range(len(AR_LCS))]
        E_drams = [dram.tile([P, n * Q], ART, name=f"E_dram_{i}")
                   for i, n in enumerate(AR_LCS)]
        denom_drams = [dram.tile([P, n * Q], ART, addr_space="Shared",
                                 name=f"denom_dram_{i}")
                       for i, n in enumerate(AR_LCS)]

        # attn working tiles, created up front: the dn loads + casts are
        # emitted INSIDE phase C right after each chunk's collective, on the
        # Scalar queue. This keeps them ahead of the NEXT chunk's collective
        # trigger in program order — otherwise the Tile semaphore accounting
        # makes the whole attn chain wait for that trigger instruction to
        # execute (which only happens ~3us after the prior mesh completes).
        attnp = bigctx.enter_context(tc.tile_pool(name="attnp", bufs=1))
        rscr = bigctx.enter_context(tc.tile_pool(name="rscr", bufs=3))
        attn_sb = attnp.tile([P, LC, Q], CDT)
        attn_flat = attn_sb.rearrange("p l q -> p (l q)")
        # attn piece width: 512 cols (2 l-chunks). The Scalar dn-load+cast
        # pipeline produces one piece per ~1.15us while av consumes one per
        # ~2.3us, so av never starves mid-chunk (1024-col pieces made the
        # pipeline only break even, costing ~2.5us stalls per chunk).
        CH = 512

        def chunk_pieces(n_lc):
            # first piece 256 cols (1 l-chunk) for the shortest possible
            # denominator->first-av lead-in, then 512-col pieces
            cols = n_lc * Q
            pieces, off = [], 0
            while off < cols:
                w = min(Q if off == 0 else CH, cols - off)
                pieces.append((off, w))
                off += w
            return pieces

        r32_tiles = {}  # (ar_i, off) -> r32 tile with the cast denominator

        ps4_ctx = ExitStack()
        ps4 = ps4_ctx.enter_context(
            tc.tile_pool(name="ps4", bufs=4, space="PSUM"))
        for sl in range(LC // SLAB):
            kT_t = kT_pre[sl]
            for s in range(SLAB):
                lc = sl * SLAB + s
                ps_s = ps4.tile([P, Q], F32, tag="ps")
                for dc in range(DC):
                    nc.tensor.matmul(
                        ps_s,
                        kT_t[:, dc, s * P:(s + 1) * P],
                        qkT_sb[:, dc, :],
                        start=(dc == 0), stop=False,
                    )
                nc.tensor.matmul(
                    ps_s, ones_row[:, :P], qkb_sb,
                    start=False, stop=True,
                )
                nc.scalar.activation(
                    out=E_sb[:, lc * Q:(lc + 1) * Q], in_=ps_s,
                    func=mybir.ActivationFunctionType.Exp, scale=SCALE,
                )
                # chunk boundary: store this AR chunk's E and trigger its
                # AllReduce immediately (the CC queue drains chunks
                # back-to-back once the ~80us ncfw wall passes).
                if lc + 1 in ar_ends:
                    ar_i = ar_ends.index(lc + 1)
                    g0 = ar_starts[ar_i] * Q
                    W = AR_LCS[ar_i] * Q
                    nc.sync.dma_start(
                        out=E_drams[ar_i],
                        in_=E_sb[:, g0:g0 + W],
                    )
                    nc.gpsimd.collective_compute(
                        "AllReduce", mybir.AluOpType.add,
                        replica_groups=[list(range(N_CORES))],
                        ins=[E_drams[ar_i].opt()],
                        outs=[denom_drams[ar_i].opt()],
                    )
                    # dn piece loads + fp16->f32 casts for THIS chunk, on
                    # the Scalar queue (idle once phase C's exps retire; the
                    # triggers just wait there for the mesh's output).
                    for off, w in chunk_pieces(AR_LCS[ar_i]):
                        dn = rscr.tile([P, CH], ART, tag="dn")
                        nc.scalar.dma_start(
                            out=dn[:, :w],
                            in_=denom_drams[ar_i][:, off:off + w])
                        r32 = rscr.tile([P, CH], F32, tag="r32")
                        nc.scalar.copy(r32[:, :w], dn[:, :w])
                        r32_tiles[(ar_i, off)] = r32
                    if ar_i == len(AR_LCS) - 1:
                        # v + Wv prefetch (10MB), deferred until the LAST
                        # chunk's E store (~62us): ncfw's warmup (~45-85us)
                        # appears to contend with HBM traffic — runs with a
                        # busy 45-75us window drew walls of 85-113us vs
                        # 61-70us with a quiet one. v lands ~90us, well
                        # before av needs it (~115us).
                        for vq in range(4):
                            nc.sync.dma_start(
                                out=v_all[:, vq * 8:(vq + 1) * 8, :],
                                in_=v_pv[:, vq * 8:(vq + 1) * 8, :])
                        nc.sync.dma_start(out=wvT_sb, in_=wvT_v)
        ps4_ctx.close()

        # ====== phases E+F interleaved per AR chunk: attn then av_T ======
        # attn = E * recip(denom). As soon as one AR chunk's denominator
        # lands (cast to f32 by the Scalar queue above), Vector runs
        # recip+mult per piece and the av_T matmuls follow -- overlapping
        # the remaining AllReduce chunk.
        with tc.tile_pool(name="accump", bufs=1, space="PSUM") as accump:
            # dc=7 shares its 2KB bank with the rowsum accumulator: PSUM
            # start=True zero-fills the WHOLE bank, so av_ps[7]'s chain
            # start (lc==0) also zeroes the rs region; the final rs matmul
            # then accumulates with start=False onto it. This keeps the rs
            # matmul free of the WAR dependency on the av-bank copies that
            # a separate rsp pool would have (saved ~2us of PE idle).
            av_ps = [accump.tile([P, Q], F32, name=f"av_ps_{dc}")
                     for dc in range(DC - 1)]
            av7rs = accump.tile([P, 2 * Q], F32, name="av_ps_7_rs")
            av_ps.append(av7rs[:, 0:Q])
            rs_ps = av7rs[:1, Q:Q + Q]
            for ar_i, n_lc in enumerate(AR_LCS):
                for off, w in chunk_pieces(n_lc):
                    g = ar_starts[ar_i] * Q + off
                    sli = slice(g, g + w)
                    r32 = r32_tiles[(ar_i, off)]
                    nc.vector.reciprocal_approx_fast(r32[:, :w], r32[:, :w])
                    nc.vector.tensor_tensor(attn_flat[:, sli], E_sb[:, sli],
                                            r32[:, :w],
                                            op=mybir.AluOpType.mult)
                for lc in range(ar_starts[ar_i], ar_ends[ar_i]):
                    at = attn_sb[:, lc, :]
                    for dc in range(DC):
                        nc.tensor.matmul(
                            av_ps[dc], v_all[:, lc, dc * P:(dc + 1) * P], at,
                            start=(lc == 0),
                            stop=(lc == LC - 1),
                        )
                    # rowsum(attn) partials accumulate on Vector (bf16):
                    # keeps 32 rider matmuls off the PE's critical path.
                    if lc == 0:
                        nc.vector.tensor_copy(rs_acc, at)
                    else:
                        nc.vector.tensor_tensor(rs_acc, rs_acc, at,
                                                op=mybir.AluOpType.add)
            # collapse rowsum partials across partitions with one matmul.
            # start=False: the region was zeroed by av_ps[7]'s chain start.
            nc.tensor.matmul(rs_ps, ones_col, rs_acc,
                             start=False, stop=True)
            nc.vector.tensor_copy(rs_sb, rs_ps)
            # split the PSUM->SBUF copies across two engines AND by q-half
            # (out-proj iterates qm outer, so its qm=0 chains start after
            # only the first 8 half-copies)
            for qm in range(Q // P):
                for dc in range(DC):
                    dst = avT_sb[:, dc, qm * P:(qm + 1) * P]
                    src = av_ps[dc][:, qm * P:(qm + 1) * P]
                    if dc % 2 == 0:
                        nc.vector.tensor_copy(dst, src)
                    else:
                        nc.scalar.copy(dst, src)

        # ================ phase G: out projection ===============
        with (tc.tile_pool(name="outp", bufs=2, space="PSUM") as outp,
              tc.tile_pool(name="outsb", bufs=2) as outsb):
            # out[q,e] = sum_d av_T[d, q-chunk].T @ WvT[d, e] + rs * bv
            NB = D // 512
            for qm in range(Q // P):
                for eb in range(NB):
                    ps_o = outp.tile([P, 512], F32, tag="ps_out")
                    for dc in range(DC):
                        nc.tensor.matmul(
                            ps_o,
                            avT_sb[:, dc, qm * P:(qm + 1) * P],
                            wvT_sb[:, dc, eb * 512:(eb + 1) * 512],
                            start=(dc == 0), stop=False,
                        )
                    nc.tensor.matmul(
                        ps_o,
                        rs_sb[:, qm * P:(qm + 1) * P],
                        bv_sb[:, eb * 512:(eb + 1) * 512],
                        start=False, stop=True,
                    )
                    o_sb = outsb.tile([P, 512], CDT, tag="o_sb")
                    nc.vector.tensor_copy(o_sb, ps_o)
                    nc.sync.dma_start(
                        out=out_v[:, qm, eb * 512:(eb + 1) * 512], in_=o_sb)
        bigctx.close()
        kslab_ctx.close()

    nc.compile()
    return nc


def _prep_inputs(q, k, v, Wq, bq, Wk, bk, Wv, bv):
    """Shard + pre-transpose + cast on host. Returns in_maps for 8 cores."""
    cnp = _np_cdt()
    f32 = np.float32

    def c(x):
        return np.ascontiguousarray(np.asarray(x, dtype=f32), dtype=cnp)

    # shared across cores
    wqT = c(np.asarray(Wq, dtype=f32).T)
    wk_ = c(Wk)
    wvT = c(np.asarray(Wv, dtype=f32).T)
    bq_ = c(np.asarray(bq, dtype=f32).reshape(1, D))
    bk_ = c(np.asarray(bk, dtype=f32).reshape(EC, P).T)
    bv_ = c(np.asarray(bv, dtype=f32).reshape(1, D))
    ones_r = np.ones((1, Q), dtype=cnp)
    ones_c = np.ones((P, 1), dtype=cnp)

    in_maps = []
    for b in range(B):
        in_maps.append({
            "kT": c(np.asarray(k[b], dtype=f32).T),
            "v_in": c(v[b]),
            "qT": c(np.asarray(q[b, :Q], dtype=f32).T),
            "wqT": wqT,
            "wk": wk_,
            "wvT": wvT,
            "bq_in": bq_,
            "bk_in": bk_,
            "bv_in": bv_,
            "ones_r": ones_r,
            "ones_c": ones_c,
        })
    return in_maps


def kernel(q, k, v, Wq, bq, Wk, bk, Wv, bv, _trace=False):
    q = np.asarray(q)
    k = np.asarray(k)
    v = np.asarray(v)
    if "nc" not in _cached:
        _cached["nc"] = build_kernel()
    nc = _cached["nc"]
    in_maps = _prep_inputs(q, k, v, Wq, bq, Wk, bk, Wv, bv)
    res = bass_utils.run_bass_kernel_spmd(
        nc, in_maps, core_ids=list(range(N_CORES)), trace=_trace)
    out = np.stack([res.results[c]["out"] for c in range(N_CORES)], axis=0)
    if _trace:
        _cached["last_results"] = res
    return out.astype(np.float32)


if __name__ == "__main__":
    rng = np.random.default_rng(0)
    ins = {
        "q": rng.standard_normal((B, L, D)).astype(np.float32),
        "k": rng.standard_normal((B, L, D)).astype(np.float32),
        "v": rng.standard_normal((B, L, D)).astype(np.float32),
        "Wq": (rng.standard_normal((D, D)) * 0.02).astype(np.float32),
        "bq": (rng.standard_normal(D) * 0.02).astype(np.float32),
        "bk": (rng.standard_normal(D) * 0.02).astype(np.float32),
        "Wk": (rng.standard_normal((D, D)) * 0.02).astype(np.float32),
        "Wv": (rng.standard_normal((D, D)) * 0.02).astype(np.float32),
        "bv": (rng.standard_normal(D) * 0.02).astype(np.float32),
    }
    out = kernel(**ins)
    print("out", out.shape, out.dtype)


# revision 27
# speedup vs baseline: 1.0365x; 1.0365x over previous
"""Trainium2 Bass kernel for BottleNeck attention (8 NeuronCores).

Reference computation (jax, fp32):
    qp = q @ Wq.T + bq          [B=8, L=4096, D=1024]
    kp = k @ Wk.T + bk
    vp = v @ Wv.T + bv
    score = qp[:, :256] @ kp.T / sqrt(D)        [B, 256, L]
    attn  = softmax(score, axis=0)              (softmax over the BATCH axis!)
    out   = attn @ vp                           [B, 256, D]

Strategy:
  * Data-parallel over batch: core c owns batch b=c.
  * Algebraic reassociation avoids projecting full-length k/v:
        qp_T = Wq.T.T @ q.T                     [D, Q]     (per batch)
        qk_T = Wk.T @ qp_T                      [D, Q]
        score_T = k.T.T @ qk_T (+ bias row)     [L, Q]
        E = exp(score_T / 32)
        denom = AllReduce_batch(E)              (axis-0 softmax denominator)
        attn_T = E / denom                      [L, Q]
        av_T = v.T-chunks @ attn_T              [D, Q]
        out = av_T.T @ Wv.T + rowsum(attn) * bv [Q, D]
    This cuts FLOPs ~3x vs projecting kp/vp at full length.
  * Host pre-transposes (k.T, q.T, Wq.T, Wv.T) so no transposes on device.
  * Compute dtype bf16 (fp32 PSUM accumulation); the AllReduce payload is
    fp16 (exp scores are positive and bounded, so fp16 rounding ~2.4e-4).

Scheduling (measured on this fleet):
  * ncfw/TOPSP collectives cannot BEGIN their mesh until the firmware is
    warm: the CC processes the first trigger somewhere in ~60-85us after
    NEFF start (run-to-run jitter, uncorrelated with trigger time), the
    mesh begins exactly 11.1us after that, and its first sync completes
    when the SLOWEST core's firmware is in (~84-113us observed). All local
    compute before that wall is free, so phases A/B/C (~60us: projections,
    scores, exps) fill the pre-wall window with the contraction index
    outermost and PSUM-resident chains.
  * The mesh has a ~17.5us FIXED cost per collective chunk (a 0.13MB
    chunk's data events take 16.6us vs 24.5us for 1.31MB, ~16us/MB
    marginal), so few big chunks win: E-AllReduce in 2 chunks [20, 12]
    l-chunks, each stored + triggered as soon as phase C finishes it (all
    triggers land well before the wall, so the CC drains them
    back-to-back). An fp8e4 wire was tried (exec 182us) but the mesh
    accumulates in the wire dtype and rel err hit 2.6e-2 (> 2e-2 gate).
  * Phase F critical path after a chunk's denominator lands: dn load +
    fp16->f32 cast on the Scalar queue -> reciprocal_approx_fast + E*recip
    mult on Vector -> av matmuls (~3.5-4.4us lead-in). The dn loads/casts
    are emitted INSIDE phase C, right after their own chunk's collective
    and BEFORE the next chunk's: Tile's semaphore accounting otherwise
    makes the attn chain wait for the next chunk's trigger instruction,
    which only executes ~3us after the prior mesh completes (this false
    dependency cost the old layout ~8-20us of lead-in).
  * rowsum(attn) is accumulated on Vector (bf16 adds per l-chunk) and
    reduced across partitions with a single 0.24us PE matmul at the end --
    this removes 32 rider matmuls (~7us) from the PE's critical path and
    lets av use 8 clean PSUM banks.
  * kT fully prefetched up front; v (8MB) + Wv DMAs deferred until the
    first chunk's E store so the AllReduce windows run on quiet HBM.
  * Post-denom0 critical path is now ~58us: lead-in 4.4 + av 37 (256
    matmuls at ~145ns, PE-bound, <7us of bubbles) + rowsum/copies 2 +
    out-proj 10 + drain 2.4. Exec = denom0 + ~58us; denom0 = wall + 26.5.
"""

import sys
from contextlib import ExitStack

sys.path.insert(0, "/opt/trn_rl_repo")

import numpy as np

import concourse.bass as bass
import concourse.mybir as mybir
import concourse.tile as tile
from concourse import bacc, bass_utils

B = 8
L = 4096
D = 1024
Q = 256  # bottleneck
N_CORES = 8
P = 128
DC = D // P  # 8 d-chunks
EC = D // P  # 8 e-chunks
LC = L // P  # 32 l-chunks
SCALE = 1.0 / 32.0  # 1/sqrt(1024)

# compute dtype for matmul operands ("bf16" | "fp32r" | "fp32")
COMPUTE = "bf16"

# AllReduce chunk sizes in l-chunks. The mesh has a ~17.5us FIXED cost per
# chunk (measured: a 0.13MB chunk's data events take 16.6us vs 24.5us for
# 1.31MB), so few big chunks win; [20,12] balances the first denominator's
# arrival against the exposed last-chunk tail.
AR_LCS = [20, 12]

_CDT = {
    "bf16": mybir.dt.bfloat16,
    "fp32r": mybir.dt.float32r,  # fp32 bits; PE rounds internally (~tf32)
    "fp32": mybir.dt.float32,
}

_cached = {}


def _np_cdt():
    if COMPUTE == "bf16":
        import ml_dtypes

        return np.dtype(ml_dtypes.bfloat16)
    return np.dtype(np.float32)


def build_kernel():
    CDT = _CDT[COMPUTE]
    F32 = mybir.dt.float32

    nc = bacc.Bacc("TRN2", target_bir_lowering=False, debug=False,
                   num_devices=N_CORES)

    # ---- per-core external inputs (host pre-transposed / pre-cast) ----
    kT = nc.dram_tensor("kT", [D, L], CDT, kind="ExternalInput")       # k[b].T
    v_in = nc.dram_tensor("v_in", [L, D], CDT, kind="ExternalInput")   # v[b]
    qT = nc.dram_tensor("qT", [D, Q], CDT, kind="ExternalInput")       # q[b,:Q].T
    wqT = nc.dram_tensor("wqT", [D, D], CDT, kind="ExternalInput")     # Wq.T
    wk = nc.dram_tensor("wk", [D, D], CDT, kind="ExternalInput")       # Wk
    wvT = nc.dram_tensor("wvT", [D, D], CDT, kind="ExternalInput")     # Wv.T
    bq_in = nc.dram_tensor("bq_in", [1, D], CDT, kind="ExternalInput")
    bk_in = nc.dram_tensor("bk_in", [P, EC], CDT, kind="ExternalInput")  # bk.reshape(EC,P).T
    bv_in = nc.dram_tensor("bv_in", [1, D], CDT, kind="ExternalInput")
    ones_r_in = nc.dram_tensor("ones_r", [1, Q], CDT, kind="ExternalInput")
    ones_c_in = nc.dram_tensor("ones_c", [P, 1], CDT, kind="ExternalInput")
    out_ext = nc.dram_tensor("out", [Q, D], CDT, kind="ExternalOutput")

    # DRAM views with the partition-chunk structure we DMA through
    kT_v = kT.rearrange("(c p) l -> p c l", p=P)        # [128, 8, 4096]
    wqT_v = wqT.rearrange("(c p) e -> p c e", p=P)      # [128, 8, 1024]
    wk_v = wk.rearrange("(c p) d -> p c d", p=P)
    wvT_v = wvT.rearrange("(c p) e -> p c e", p=P)
    qT_v = qT.rearrange("(c p) q -> p c q", p=P)        # [128, 8, 256]
    out_v = out_ext.rearrange("(m p) e -> p m e", p=P)  # [128, 2, 1024]

    with tile.TileContext(nc) as tc, ExitStack() as top:
        consts = top.enter_context(tc.tile_pool(name="consts", bufs=1))
        qstate = top.enter_context(tc.tile_pool(name="qstate", bufs=1))
        dram = top.enter_context(tc.tile_pool(name="dram", bufs=1, space="DRAM"))

        # ---------------- constants ----------------
        ones_row = consts.tile([1, Q], CDT)       # [1, 256] of 1.0
        ones_col = consts.tile([P, 1], CDT)       # [128, 1] of 1.0
        nc.sync.dma_start(out=ones_row, in_=ones_r_in[:, :])
        nc.sync.dma_start(out=ones_col, in_=ones_c_in[:, :])
        bq_sb = consts.tile([1, D], CDT)
        bk_sb = consts.tile([P, EC], CDT)
        bv_sb = consts.tile([1, D], CDT)
        nc.sync.dma_start(out=bq_sb, in_=bq_in[:, :])
        nc.sync.dma_start(out=bk_sb, in_=bk_in[:, :])
        nc.sync.dma_start(out=bv_sb, in_=bv_in[:, :])

        ART = mybir.dt.float16  # AllReduce payload dtype (E fits fp16 range)

        # Scalar-queue warmup: a dummy activation with no upstream compute
        # deps loads the EXP table and wakes the Scalar queue at ~7us, so
        # the first real EXP fires at data-readiness instead of ~50us.
        warm = consts.tile([1, Q], ART)
        nc.scalar.activation(out=warm, in_=ones_row,
                             func=mybir.ActivationFunctionType.Exp,
                             scale=SCALE)

        qpT_sb = qstate.tile([P, EC, Q], CDT)
        qkT_sb = qstate.tile([P, DC, Q], CDT)
        qkb_sb = qstate.tile([1, Q], CDT)
        avT_sb = qstate.tile([P, DC, Q], CDT)
        rs_acc = qstate.tile([P, Q], CDT)   # rowsum(attn) partial, per part.
        rs_sb = qstate.tile([1, Q], CDT)

        SLAB = 4  # l-chunks per kT slab (512 l positions)
        kslab_ctx = ExitStack()
        kslabs = kslab_ctx.enter_context(tc.tile_pool(name="kslabs", bufs=6))

        # ================ phases A+B: q-side projections ================
        # Contraction index outermost with all PSUM chains resident: the
        # first matmuls need only the first 128-row chunk of the weight.
        # Each phase is split in two halves to bound live PSUM.
        with tc.tile_pool(name="wab", bufs=1) as wab, \
             tc.tile_pool(name="psAB", bufs=4, space="PSUM") as psAB, \
             tc.tile_pool(name="psbias", bufs=1, space="PSUM") as psbias:
            wqT_sb = wab.tile([P, EC, D], CDT)
            wk_sb = wab.tile([P, EC, D], CDT)
            qT_sb = wab.tile([P, DC, Q], CDT)
            # DMA order = consumption order: per-dc (wqT,qT) for A, then
            # the first kT slab, then per-ec wk for B, second kT slab.
            for dc in range(DC):
                nc.sync.dma_start(out=wqT_sb[:, dc, :], in_=wqT_v[:, dc, :])
                nc.sync.dma_start(out=qT_sb[:, dc, :], in_=qT_v[:, dc, :])
            kT_pre = []
            for sl in range(2):
                kT_t = kslabs.tile([P, DC, SLAB * P], CDT, tag="kT",
                                   name=f"kT_pre{sl}")
                nc.sync.dma_start(
                    out=kT_t, in_=kT_v[:, :, sl * SLAB * P:(sl + 1) * SLAB * P])
                kT_pre.append(kT_t)
            for ec in range(EC):
                nc.sync.dma_start(out=wk_sb[:, ec, :], in_=wk_v[:, ec, :])
            # prefetch the REMAINING kT slabs now (8 bufs, no rotation):
            # their DMAs must issue before the deferred v prefetch so the
            # later AllReduce chunks' scores are never DMA-starved.
            for sl in range(2, LC // SLAB):
                kT_t = kslabs.tile([P, DC, SLAB * P], CDT, tag="kT",
                                   name=f"kT_pre{sl}")
                nc.sync.dma_start(
                    out=kT_t, in_=kT_v[:, :, sl * SLAB * P:(sl + 1) * SLAB * P])
                kT_pre.append(kT_t)

            # phase A: qp_T[e,q] = sum_d WqT[d, e-chunk].T @ qT[d, q] + bq
            for half in range(2):
                ecs = range(half * 4, half * 4 + 4)
                psA = [psAB.tile([P, Q], F32, tag="ab", name=f"psA_{half}_{i}")
                       for i in range(4)]
                for dc in range(DC):
                    for i, ec in enumerate(ecs):
                        nc.tensor.matmul(
                            psA[i],
                            wqT_sb[:, dc, ec * P:(ec + 1) * P],
                            qT_sb[:, dc, :],
                            start=(dc == 0), stop=False,
                        )
                for i, ec in enumerate(ecs):
                    nc.tensor.matmul(
                        psA[i], bq_sb[:, ec * P:(ec + 1) * P], ones_row,
                        start=False, stop=True,
                    )
                    nc.vector.tensor_copy(qpT_sb[:, ec, :], psA[i])

            # phase B: qk_T[d,q] = sum_e Wk[e, d-chunk].T @ qp_T[e, q]
            ps_qkb = psbias.tile([1, Q], F32, name="ps_qkb")
            for half in range(2):
                dcs = range(half * 4, half * 4 + 4)
                psB = [psAB.tile([P, Q], F32, tag="ab", name=f"psB_{half}_{i}")
                       for i in range(4)]
                for ec in range(EC):
                    for i, dc in enumerate(dcs):
                        nc.tensor.matmul(
                            psB[i],
                            wk_sb[:, ec, dc * P:(dc + 1) * P],
                            qpT_sb[:, ec, :],
                            start=(ec == 0), stop=(ec == EC - 1),
                        )
                for i, dc in enumerate(dcs):
                    nc.vector.tensor_copy(qkT_sb[:, dc, :], psB[i])
            # score bias row: qkb[q] = sum_e bk[e] * qp_T[e, q]
            for ec in range(EC):
                nc.tensor.matmul(
                    ps_qkb, bk_sb[:, ec:ec + 1], qpT_sb[:, ec, :],
                    start=(ec == 0), stop=(ec == EC - 1),
                )
            nc.vector.tensor_copy(qkb_sb, ps_qkb)

        # ================ phase C: score_T -> E -> DRAM ================
        # NOTE: an fp8e4 AllReduce wire was tried (halves mesh bytes, exec
        # 182us) but the mesh accumulates in the wire dtype and the compound
        # rounding pushed rel err to 2.6e-2 (> the 2e-2 gate). fp16 it is.
        bigctx = ExitStack()
        bigbuf = bigctx.enter_context(tc.tile_pool(name="bigbuf", bufs=1))
        wvp = bigctx.enter_context(tc.tile_pool(name="wvp", bufs=1))
        E_sb = bigbuf.tile([P, LC * Q], ART)          # [128, 8192]
        wvT_sb = wvp.tile([P, DC, D], CDT)
        v_all = bigbuf.tile([P, LC, D], CDT)
        v_pv = v_in.rearrange("(c p) d -> p c d", p=P)  # [128, 32, 1024]
        assert sum(AR_LCS) == LC
        ar_starts = [sum(AR_LCS[:i]) for i in range(len(AR_LCS))]
        ar_ends = [ar_starts[i] + AR_LCS[i] for i in range(len(AR_LCS))]
        E_drams = [dram.tile([P, n * Q], ART, name=f"E_dram_{i}")
                   for i, n in enumerate(AR_LCS)]
        denom_drams = [dram.tile([P, n * Q], ART, addr_space="Shared",
                                 name=f"denom_dram_{i}")
                       for i, n in enumerate(AR_LCS)]

        # attn working tiles, created up front: the dn loads + casts are
        # emitted INSIDE phase C right after each chunk's collective, on the
        # Scalar queue. This keeps them ahead of the NEXT chunk's collective
        # trigger in program order — otherwise the Tile semaphore accounting
        # makes the whole attn chain wait for that trigger instruction to
        # execute (which only happens ~3us after the prior mesh completes).
        attnp = bigctx.enter_context(tc.tile_pool(name="attnp", bufs=1))
        rscr = bigctx.enter_context(tc.tile_pool(name="rscr", bufs=3))
        attn_sb = attnp.tile([P, LC, Q], CDT)
        attn_flat = attn_sb.rearrange("p l q -> p (l q)")
        # attn piece width: 512 cols (2 l-chunks). The Scalar dn-load+cast
        # pipeline produces one piece per ~1.15us while av consumes one per
        # ~2.3us, so av never starves mid-chunk (1024-col pieces made the
        # pipeline only break even, costing ~2.5us stalls per chunk).
        CH = 512

        def chunk_pieces(n_lc):
            # first piece 256 cols (1 l-chunk) for the shortest possible
            # denominator->first-av lead-in, then 512-col pieces
            cols = n_lc * Q
            pieces, off = [], 0
            while off < cols:
                w = min(Q if off == 0 else CH, cols - off)
                pieces.append((off, w))
                off += w
            return pieces

        r32_tiles = {}  # (ar_i, off) -> r32 tile with the cast denominator

        ps4_ctx = ExitStack()
        ps4 = ps4_ctx.enter_context(
            tc.tile_pool(name="ps4", bufs=4, space="PSUM"))
        for sl in range(LC // SLAB):
            kT_t = kT_pre[sl]
            for s in range(SLAB):
                lc = sl * SLAB + s
                ps_s = ps4.tile([P, Q], F32, tag="ps")
                for dc in range(DC):
                    nc.tensor.matmul(
                        ps_s,
                        kT_t[:, dc, s * P:(s + 1) * P],
                        qkT_sb[:, dc, :],
                        start=(dc == 0), stop=False,
                    )
                nc.tensor.matmul(
                    ps_s, ones_row[:, :P], qkb_sb,
                    start=False, stop=True,
                )
                nc.scalar.activation(
                    out=E_sb[:, lc * Q:(lc + 1) * Q], in_=ps_s,
                    func=mybir.ActivationFunctionType.Exp, scale=SCALE,
                )
                # chunk boundary: store this AR chunk's E and trigger its
                # AllReduce immediately (the CC queue drains chunks
                # back-to-back once the ~80us ncfw wall passes).
                if lc + 1 in ar_ends:
                    ar_i = ar_ends.index(lc + 1)
                    g0 = ar_starts[ar_i] * Q
                    W = AR_LCS[ar_i] * Q
                    nc.sync.dma_start(
                        out=E_drams[ar_i],
                        in_=E_sb[:, g0:g0 + W],
                    )
                    nc.gpsimd.collective_compute(
                        "AllReduce", mybir.AluOpType.add,
                        replica_groups=[list(range(N_CORES))],
                        ins=[E_drams[ar_i].opt()],
                        outs=[denom_drams[ar_i].opt()],
                    )
                    # dn piece loads + fp16->f32 casts for THIS chunk, on
                    # the Scalar queue (idle once phase C's exps retire; the
                    # triggers just wait there for the mesh's output).
                    for off, w in chunk_pieces(AR_LCS[ar_i]):
                        dn = rscr.tile([P, CH], ART, tag="dn")
                        nc.scalar.dma_start(
                            out=dn[:, :w],
                            in_=denom_drams[ar_i][:, off:off + w])
                        r32 = rscr.tile([P, CH], F32, tag="r32")
                        nc.scalar.copy(r32[:, :w], dn[:, :w])
                        r32_tiles[(ar_i, off)] = r32
                    if ar_i == len(AR_LCS) - 1:
                        # v + Wv prefetch (10MB), deferred until the LAST
                        # chunk's E store (~62us): ncfw's warmup (~45-85us)
                        # appears to contend with HBM traffic — runs with a
                        # busy 45-75us window drew walls of 85-113us vs
                        # 61-70us with a quiet one. v lands ~90us, well
                        # before av needs it (~115us).
                        for vq in range(4):
                            nc.sync.dma_start(
                                out=v_all[:, vq * 8:(vq + 1) * 8, :],
                                in_=v_pv[:, vq * 8:(vq + 1) * 8, :])
                        nc.sync.dma_start(out=wvT_sb, in_=wvT_v)
        ps4_ctx.close()

        # ====== phases E+F interleaved per AR chunk: attn then av_T ======
        # attn = E * recip(denom). As soon as one AR chunk's denominator
        # lands (cast to f32 by the Scalar queue above), Vector runs
        # recip+mult per piece and the av_T matmuls follow -- overlapping
        # the remaining AllReduce chunk.
        with tc.tile_pool(name="accump", bufs=1, space="PSUM") as accump:
            # dc=7 shares its 2KB bank with the rowsum accumulator: PSUM
            # start=True zero-fills the WHOLE bank, so av_ps[7]'s chain
            # start (lc==0) also zeroes the rs region; the final rs matmul
            # then accumulates with start=False onto it. This keeps the rs
            # matmul free of the WAR dependency on the av-bank copies that
            # a separate rsp pool would have (saved ~2us of PE idle).
            av_ps = [accump.tile([P, Q], F32, name=f"av_ps_{dc}")
                     for dc in range(DC - 1)]
            av7rs = accump.tile([P, 2 * Q], F32, name="av_ps_7_rs")
            av_ps.append(av7rs[:, 0:Q])
            rs_ps = av7rs[:1, Q:Q + Q]
            for ar_i, n_lc in enumerate(AR_LCS):
                for off, w in chunk_pieces(n_lc):
                    g = ar_starts[ar_i] * Q + off
                    sli = slice(g, g + w)
                    r32 = r32_tiles[(ar_i, off)]
                    nc.vector.reciprocal_approx_fast(r32[:, :w], r32[:, :w])
                    nc.vector.tensor_tensor(attn_flat[:, sli], E_sb[:, sli],
                                            r32[:, :w],
                                            op=mybir.AluOpType.mult)
                for lc in range(ar_starts[ar_i], ar_ends[ar_i]):
                    at = attn_sb[:, lc, :]
                    for dc in range(DC):
                        nc.tensor.matmul(
                            av_ps[dc], v_all[:, lc, dc * P:(dc + 1) * P], at,
                            start=(lc == 0),
                            stop=(lc == LC - 1),
                        )
                    # rowsum(attn) partials accumulate on Vector (bf16):
                    # keeps 32 rider matmuls off the PE's critical path.
                    if lc == 0:
                        nc.vector.tensor_copy(rs_acc, at)
                    else:
                        nc.vector.tensor_tensor(rs_acc, rs_acc, at,
                                                op=mybir.AluOpType.add)
            # collapse rowsum partials across partitions with one matmul.
            # start=False: the region was zeroed by av_ps[7]'s chain start.
            nc.tensor.matmul(rs_ps, ones_col, rs_acc,
                             start=False, stop=True)
            nc.vector.tensor_copy(rs_sb, rs_ps)
            # split the PSUM->SBUF copies across two engines AND by q-half
            # (out-proj iterates qm outer, so its qm=0 chains start after
            # only the first 8 half-copies)
            for qm in range(Q // P):
                for dc in range(DC):
                    dst = avT_sb[:, dc, qm * P:(qm + 1) * P]
                    src = av_ps[dc][:, qm * P:(qm + 1) * P]
                    if dc % 2 == 0:
                        nc.vector.tensor_copy(dst, src)
                    else:
                        nc.scalar.copy(dst, src)

        # ================ phase G: out projection ===============
        with (tc.tile_pool(name="outp", bufs=2, space="PSUM") as outp,
              tc.tile_pool(name="outsb", bufs=2) as outsb):
            # out[q,e] = sum_d av_T[d, q-chunk].T @ WvT[d, e] + rs * bv
            # The two eb chains are INTERLEAVED per dc: consecutive matmuls
            # then alternate PSUM banks, so each matmul's accumulator drain
            # and the next LDWEIGHTS overlap the sibling chain's matmul
            # (a single chain serializes at 0.37us/matmul vs 0.28 here),
            # and both chains share the same stationary avT chunk per dc.
            NB = D // 512
            for qm in range(Q // P):
                ps_o = [outp.tile([P, 512], F32, tag=f"ps_out{eb}",
                                  name=f"ps_o_{qm}_{eb}")
                        for eb in range(NB)]
                for dc in range(DC):
                    for eb in range(NB):
                        nc.tensor.matmul(
                            ps_o[eb],
                            avT_sb[:, dc, qm * P:(qm + 1) * P],
                            wvT_sb[:, dc, eb * 512:(eb + 1) * 512],
                            start=(dc == 0), stop=False,
                        )
                for eb in range(NB):
                    nc.tensor.matmul(
                        ps_o[eb],
                        rs_sb[:, qm * P:(qm + 1) * P],
                        bv_sb[:, eb * 512:(eb + 1) * 512],
                        start=False, stop=True,
                    )
                    o_sb = outsb.tile([P, 512], CDT, tag="o_sb")
                    nc.vector.tensor_copy(o_sb, ps_o[eb])
                    nc.sync.dma_start(
                        out=out_v[:, qm, eb * 512:(eb + 1) * 512], in_=o_sb)
        bigctx.close()
        kslab_ctx.close()

    nc.compile()
    return nc


def _prep_inputs(q, k, v, Wq, bq, Wk, bk, Wv, bv):
    """Shard + pre-transpose + cast on host. Returns in_maps for 8 cores."""
    cnp = _np_cdt()
    f32 = np.float32

    def c(x):
        return np.ascontiguousarray(np.asarray(x, dtype=f32), dtype=cnp)

    # shared across cores
    wqT = c(np.asarray(Wq, dtype=f32).T)
    wk_ = c(Wk)
    wvT = c(np.asarray(Wv, dtype=f32).T)
    bq_ = c(np.asarray(bq, dtype=f32).reshape(1, D))
    bk_ = c(np.asarray(bk, dtype=f32).reshape(EC, P).T)
    bv_ = c(np.asarray(bv, dtype=f32).reshape(1, D))
    ones_r = np.ones((1, Q), dtype=cnp)
    ones_c = np.ones((P, 1), dtype=cnp)

    in_maps = []
    for b in range(B):
        in_maps.append({
            "kT": c(np.asarray(k[b], dtype=f32).T),
            "v_in": c(v[b]),
            "qT": c(np.asarray(q[b, :Q], dtype=f32).T),
            "wqT": wqT,
            "wk": wk_,
            "wvT": wvT,
            "bq_in": bq_,
            "bk_in": bk_,
            "bv_in": bv_,
            "ones_r": ones_r,
            "ones_c": ones_c,
        })
    return in_maps


def kernel(q, k, v, Wq, bq, Wk, bk, Wv, bv, _trace=False):
    q = np.asarray(q)
    k = np.asarray(k)
    v = np.asarray(v)
    if "nc" not in _cached:
        _cached["nc"] = build_kernel()
    nc = _cached["nc"]
    in_maps = _prep_inputs(q, k, v, Wq, bq, Wk, bk, Wv, bv)
    res = bass_utils.run_bass_kernel_spmd(
        nc, in_maps, core_ids=list(range(N_CORES)), trace=_trace)
    out = np.stack([res.results[c]["out"] for c in range(N_CORES)], axis=0)
    if _trace:
        _cached["last_results"] = res
    return out.astype(np.float32)


if __name__ == "__main__":
    rng = np.random.default_rng(0)
    ins = {
        "q": rng.standard_normal((B, L, D)).astype(np.float32),
        "k": rng.standard_normal((B, L, D)).astype(np.float32),
        "v": rng.standard_normal((B, L, D)).astype(np.float32),
        "Wq": (rng.standard_normal((D, D)) * 0.02).astype(np.float32),
        "bq": (rng.standard_normal(D) * 0.02).astype(np.float32),
        "bk": (rng.standard_normal(D) * 0.02).astype(np.float32),
        "Wk": (rng.standard_normal((D, D)) * 0.02).astype(np.float32),
        "Wv": (rng.standard_normal((D, D)) * 0.02).astype(np.float32),
        "bv": (rng.standard_normal(D) * 0.02).astype(np.float32),
    }
    out = kernel(**ins)
    print("out", out.shape, out.dtype)


# revision 33
# speedup vs baseline: 1.0373x; 1.0008x over previous
"""Trainium2 Bass kernel for BottleNeck attention (8 NeuronCores).

Reference computation (jax, fp32):
    qp = q @ Wq.T + bq          [B=8, L=4096, D=1024]
    kp = k @ Wk.T + bk
    vp = v @ Wv.T + bv
    score = qp[:, :256] @ kp.T / sqrt(D)        [B, 256, L]
    attn  = softmax(score, axis=0)              (softmax over the BATCH axis!)
    out   = attn @ vp                           [B, 256, D]

Strategy:
  * Data-parallel over batch: core c owns batch b=c.
  * Algebraic reassociation avoids projecting full-length k/v:
        qp_T = Wq.T.T @ q.T                     [D, Q]     (per batch)
        qk_T = Wk.T @ qp_T                      [D, Q]
        score_T = k.T.T @ qk_T (+ bias row)     [L, Q]
        E = exp(score_T / 32)
        denom = AllReduce_batch(E)              (axis-0 softmax denominator)
        attn_T = E / denom                      [L, Q]
        av_T = v.T-chunks @ attn_T              [D, Q]
        out = av_T.T @ Wv.T + rowsum(attn) * bv [Q, D]
    This cuts FLOPs ~3x vs projecting kp/vp at full length.
  * Host pre-transposes (k.T, q.T, Wq.T, Wv.T) so no transposes on device.
  * Compute dtype bf16 (fp32 PSUM accumulation); the AllReduce payload is
    fp16 (exp scores are positive and bounded, so fp16 rounding ~2.4e-4).

Scheduling (measured on this fleet):
  * ncfw/TOPSP collectives cannot BEGIN their mesh until the firmware is
    warm: the CC processes the first trigger somewhere in ~60-85us after
    NEFF start (run-to-run jitter, uncorrelated with trigger time), the
    mesh begins exactly 11.1us after that, and its first sync completes
    when the SLOWEST core's firmware is in (~84-113us observed). All local
    compute before that wall is free, so phases A/B/C (~60us: projections,
    scores, exps) fill the pre-wall window with the contraction index
    outermost and PSUM-resident chains.
  * The mesh has a ~17.5us FIXED cost per collective chunk (a 0.13MB
    chunk's data events take 16.6us vs 24.5us for 1.31MB, ~16us/MB
    marginal), so few big chunks win: E-AllReduce in 2 chunks [20, 12]
    l-chunks, each stored + triggered as soon as phase C finishes it (all
    triggers land well before the wall, so the CC drains them
    back-to-back). An fp8e4 wire was tried (exec 182us) but the mesh
    accumulates in the wire dtype and rel err hit 2.6e-2 (> 2e-2 gate).
  * Phase F critical path after a chunk's denominator lands: dn load +
    fp16->f32 cast on the Scalar queue -> reciprocal_approx_fast + E*recip
    mult on Vector -> av matmuls (~3.5-4.4us lead-in). The dn loads/casts
    are emitted INSIDE phase C, right after their own chunk's collective
    and BEFORE the next chunk's: Tile's semaphore accounting otherwise
    makes the attn chain wait for the next chunk's trigger instruction,
    which only executes ~3us after the prior mesh completes (this false
    dependency cost the old layout ~8-20us of lead-in).
  * rowsum(attn) is accumulated on Vector (bf16 adds per l-chunk) and
    reduced across partitions with a single 0.24us PE matmul at the end --
    this removes 32 rider matmuls (~7us) from the PE's critical path and
    lets av use 8 clean PSUM banks.
  * kT fully prefetched up front; v (8MB) + Wv DMAs deferred until the
    first chunk's E store so the AllReduce windows run on quiet HBM.
  * Post-denom0 critical path is now ~58us: lead-in 4.4 + av 37 (256
    matmuls at ~145ns, PE-bound, <7us of bubbles) + rowsum/copies 2 +
    out-proj 10 + drain 2.4. Exec = denom0 + ~58us; denom0 = wall + 26.5.
"""

import sys
from contextlib import ExitStack

sys.path.insert(0, "/opt/trn_rl_repo")

import numpy as np

import concourse.bass as bass
import concourse.mybir as mybir
import concourse.tile as tile
from concourse import bacc, bass_utils

B = 8
L = 4096
D = 1024
Q = 256  # bottleneck
N_CORES = 8
P = 128
DC = D // P  # 8 d-chunks
EC = D // P  # 8 e-chunks
LC = L // P  # 32 l-chunks
SCALE = 1.0 / 32.0  # 1/sqrt(1024)

# compute dtype for matmul operands ("bf16" | "fp32r" | "fp32")
COMPUTE = "bf16"

# AllReduce chunk sizes in l-chunks. The mesh has a ~17.5us FIXED cost per
# chunk (measured: a 0.13MB chunk's data events take 16.6us vs 24.5us for
# 1.31MB), so few big chunks win; [20,12] balances the first denominator's
# arrival against the exposed last-chunk tail.
AR_LCS = [20, 12]

_CDT = {
    "bf16": mybir.dt.bfloat16,
    "fp32r": mybir.dt.float32r,  # fp32 bits; PE rounds internally (~tf32)
    "fp32": mybir.dt.float32,
}

_cached = {}


def _np_cdt():
    if COMPUTE == "bf16":
        import ml_dtypes

        return np.dtype(ml_dtypes.bfloat16)
    return np.dtype(np.float32)


def build_kernel():
    CDT = _CDT[COMPUTE]
    F32 = mybir.dt.float32

    nc = bacc.Bacc("TRN2", target_bir_lowering=False, debug=False,
                   num_devices=N_CORES)

    A8 = mybir.dt.float8e4
    FP8_LCS = AR_LCS[0]  # leading l-chunks whose av runs fp8 DoubleRow
    G8 = FP8_LCS // 2    # DoubleRow processes l-chunk PAIRS

    # ---- per-core external inputs (host pre-transposed / pre-cast) ----
    kT = nc.dram_tensor("kT", [D, L], CDT, kind="ExternalInput")       # k[b].T
    # v for the fp8 av range, packed [p, pair, 2, d] for DoubleRow
    v8_in = nc.dram_tensor("v8_in", [P, G8, 2, D], A8, kind="ExternalInput")
    # v for the bf16 av range (l >= FP8_LCS*P)
    v_in = nc.dram_tensor("v_in", [(LC - FP8_LCS) * P, D], CDT,
                          kind="ExternalInput")
    qT = nc.dram_tensor("qT", [D, Q], CDT, kind="ExternalInput")       # q[b,:Q].T
    wqT = nc.dram_tensor("wqT", [D, D], CDT, kind="ExternalInput")     # Wq.T
    wk = nc.dram_tensor("wk", [D, D], CDT, kind="ExternalInput")       # Wk
    wvT = nc.dram_tensor("wvT", [D, D], CDT, kind="ExternalInput")     # Wv.T
    bq_in = nc.dram_tensor("bq_in", [1, D], CDT, kind="ExternalInput")
    bk_in = nc.dram_tensor("bk_in", [P, EC], CDT, kind="ExternalInput")  # bk.reshape(EC,P).T
    bv_in = nc.dram_tensor("bv_in", [1, D], CDT, kind="ExternalInput")
    ones_r_in = nc.dram_tensor("ones_r", [1, Q], CDT, kind="ExternalInput")
    ones_c_in = nc.dram_tensor("ones_c", [P, 1], CDT, kind="ExternalInput")
    out_ext = nc.dram_tensor("out", [Q, D], CDT, kind="ExternalOutput")

    # DRAM views with the partition-chunk structure we DMA through
    kT_v = kT.rearrange("(c p) l -> p c l", p=P)        # [128, 8, 4096]
    wqT_v = wqT.rearrange("(c p) e -> p c e", p=P)      # [128, 8, 1024]
    wk_v = wk.rearrange("(c p) d -> p c d", p=P)
    wvT_v = wvT.rearrange("(c p) e -> p c e", p=P)
    qT_v = qT.rearrange("(c p) q -> p c q", p=P)        # [128, 8, 256]
    out_v = out_ext.rearrange("(m p) e -> p m e", p=P)  # [128, 2, 1024]

    with tile.TileContext(nc) as tc, ExitStack() as top:
        consts = top.enter_context(tc.tile_pool(name="consts", bufs=1))
        qstate = top.enter_context(tc.tile_pool(name="qstate", bufs=1))
        dram = top.enter_context(tc.tile_pool(name="dram", bufs=1, space="DRAM"))

        # ---------------- constants ----------------
        ones_row = consts.tile([1, Q], CDT)       # [1, 256] of 1.0
        ones_col = consts.tile([P, 1], CDT)       # [128, 1] of 1.0
        nc.sync.dma_start(out=ones_row, in_=ones_r_in[:, :])
        nc.sync.dma_start(out=ones_col, in_=ones_c_in[:, :])
        bq_sb = consts.tile([1, D], CDT)
        bk_sb = consts.tile([P, EC], CDT)
        bv_sb = consts.tile([1, D], CDT)
        nc.sync.dma_start(out=bq_sb, in_=bq_in[:, :])
        nc.sync.dma_start(out=bk_sb, in_=bk_in[:, :])
        nc.sync.dma_start(out=bv_sb, in_=bv_in[:, :])

        ART = mybir.dt.float16  # AllReduce payload dtype (E fits fp16 range)

        # Scalar-queue warmup: a dummy activation with no upstream compute
        # deps loads the EXP table and wakes the Scalar queue at ~7us, so
        # the first real EXP fires at data-readiness instead of ~50us.
        warm = consts.tile([1, Q], ART)
        nc.scalar.activation(out=warm, in_=ones_row,
                             func=mybir.ActivationFunctionType.Exp,
                             scale=SCALE)

        qpT_sb = qstate.tile([P, EC, Q], CDT)
        qkT_sb = qstate.tile([P, DC, Q], CDT)
        qkb_sb = qstate.tile([1, Q], CDT)
        avT_sb = qstate.tile([P, DC, Q], CDT)
        rs_acc = qstate.tile([P, Q], CDT)   # rowsum(attn) partial, per part.
        rs_sb = qstate.tile([1, Q], CDT)

        SLAB = 4  # l-chunks per kT slab (512 l positions)
        kslab_ctx = ExitStack()
        kslabs = kslab_ctx.enter_context(tc.tile_pool(name="kslabs", bufs=6))

        # ================ phases A+B: q-side projections ================
        # Contraction index outermost with all PSUM chains resident: the
        # first matmuls need only the first 128-row chunk of the weight.
        # Each phase is split in two halves to bound live PSUM.
        with tc.tile_pool(name="wab", bufs=1) as wab, \
             tc.tile_pool(name="psAB", bufs=4, space="PSUM") as psAB, \
             tc.tile_pool(name="psbias", bufs=1, space="PSUM") as psbias:
            wqT_sb = wab.tile([P, EC, D], CDT)
            wk_sb = wab.tile([P, EC, D], CDT)
            qT_sb = wab.tile([P, DC, Q], CDT)
            # DMA order = consumption order: per-dc (wqT,qT) for A, then
            # the first kT slab, then per-ec wk for B, second kT slab.
            for dc in range(DC):
                nc.sync.dma_start(out=wqT_sb[:, dc, :], in_=wqT_v[:, dc, :])
                nc.sync.dma_start(out=qT_sb[:, dc, :], in_=qT_v[:, dc, :])
            kT_pre = []
            for sl in range(2):
                kT_t = kslabs.tile([P, DC, SLAB * P], CDT, tag="kT",
                                   name=f"kT_pre{sl}")
                nc.sync.dma_start(
                    out=kT_t, in_=kT_v[:, :, sl * SLAB * P:(sl + 1) * SLAB * P])
                kT_pre.append(kT_t)
            for ec in range(EC):
                nc.sync.dma_start(out=wk_sb[:, ec, :], in_=wk_v[:, ec, :])
            # prefetch the REMAINING kT slabs now (8 bufs, no rotation):
            # their DMAs must issue before the deferred v prefetch so the
            # later AllReduce chunks' scores are never DMA-starved.
            for sl in range(2, LC // SLAB):
                kT_t = kslabs.tile([P, DC, SLAB * P], CDT, tag="kT",
                                   name=f"kT_pre{sl}")
                nc.sync.dma_start(
                    out=kT_t, in_=kT_v[:, :, sl * SLAB * P:(sl + 1) * SLAB * P])
                kT_pre.append(kT_t)

            # phase A: qp_T[e,q] = sum_d WqT[d, e-chunk].T @ qT[d, q] + bq
            for half in range(2):
                ecs = range(half * 4, half * 4 + 4)
                psA = [psAB.tile([P, Q], F32, tag="ab", name=f"psA_{half}_{i}")
                       for i in range(4)]
                for dc in range(DC):
                    for i, ec in enumerate(ecs):
                        nc.tensor.matmul(
                            psA[i],
                            wqT_sb[:, dc, ec * P:(ec + 1) * P],
                            qT_sb[:, dc, :],
                            start=(dc == 0), stop=False,
                        )
                for i, ec in enumerate(ecs):
                    nc.tensor.matmul(
                        psA[i], bq_sb[:, ec * P:(ec + 1) * P], ones_row,
                        start=False, stop=True,
                    )
                    nc.vector.tensor_copy(qpT_sb[:, ec, :], psA[i])

            # phase B: qk_T[d,q] = sum_e Wk[e, d-chunk].T @ qp_T[e, q]
            ps_qkb = psbias.tile([1, Q], F32, name="ps_qkb")
            for half in range(2):
                dcs = range(half * 4, half * 4 + 4)
                psB = [psAB.tile([P, Q], F32, tag="ab", name=f"psB_{half}_{i}")
                       for i in range(4)]
                for ec in range(EC):
                    for i, dc in enumerate(dcs):
                        nc.tensor.matmul(
                            psB[i],
                            wk_sb[:, ec, dc * P:(dc + 1) * P],
                            qpT_sb[:, ec, :],
                            start=(ec == 0), stop=(ec == EC - 1),
                        )
                for i, dc in enumerate(dcs):
                    nc.vector.tensor_copy(qkT_sb[:, dc, :], psB[i])
            # score bias row: qkb[q] = sum_e bk[e] * qp_T[e, q]
            for ec in range(EC):
                nc.tensor.matmul(
                    ps_qkb, bk_sb[:, ec:ec + 1], qpT_sb[:, ec, :],
                    start=(ec == 0), stop=(ec == EC - 1),
                )
            nc.vector.tensor_copy(qkb_sb, ps_qkb)

        # ================ phase C: score_T -> E -> DRAM ================
        # NOTE: an fp8e4 AllReduce wire was tried (halves mesh bytes, exec
        # 182us) but the mesh accumulates in the wire dtype and the compound
        # rounding pushed rel err to 2.6e-2 (> the 2e-2 gate). fp16 it is.
        bigctx = ExitStack()
        bigbuf = bigctx.enter_context(tc.tile_pool(name="bigbuf", bufs=1))
        wvp = bigctx.enter_context(tc.tile_pool(name="wvp", bufs=1))
        E_sb = bigbuf.tile([P, LC * Q], ART)          # [128, 8192]
        wvT_sb = wvp.tile([P, DC, D], CDT)
        v8_sb = bigbuf.tile([P, G8, 2, D], A8)        # fp8 v, paired
        v_hi = bigbuf.tile([P, LC - FP8_LCS, D], CDT)  # bf16 v tail
        v_pv = v_in.rearrange("(c p) d -> p c d", p=P)  # [128, 12, 1024]
        assert sum(AR_LCS) == LC
        ar_starts = [sum(AR_LCS[:i]) for i in range(len(AR_LCS))]
        ar_ends = [ar_starts[i] + AR_LCS[i] for i in range(len(AR_LCS))]
        E_drams = [dram.tile([P, n * Q], ART, name=f"E_dram_{i}")
                   for i, n in enumerate(AR_LCS)]
        denom_drams = [dram.tile([P, n * Q], ART, addr_space="Shared",
                                 name=f"denom_dram_{i}")
                       for i, n in enumerate(AR_LCS)]

        # attn working tiles, created up front: the dn loads + casts are
        # emitted INSIDE phase C right after each chunk's collective, on the
        # Scalar queue. This keeps them ahead of the NEXT chunk's collective
        # trigger in program order — otherwise the Tile semaphore accounting
        # makes the whole attn chain wait for that trigger instruction to
        # execute (which only happens ~3us after the prior mesh completes).
        attnp = bigctx.enter_context(tc.tile_pool(name="attnp", bufs=1))
        rscr = bigctx.enter_context(tc.tile_pool(name="rscr", bufs=3))
        # chunk 0's attn in fp8 (paired layout for DoubleRow), tail in bf16
        attn8_sb = attnp.tile([P, G8, 2, Q], A8)
        attn8_flat = attn8_sb.rearrange("p g j q -> p (g j q)")
        attn_sb = attnp.tile([P, LC - FP8_LCS, Q], CDT)
        attn_flat = attn_sb.rearrange("p l q -> p (l q)")
        # attn piece width: 512 cols (2 l-chunks). The Scalar dn-load+cast
        # pipeline produces one piece per ~1.15us while av consumes one per
        # ~2.3us, so av never starves mid-chunk (1024-col pieces made the
        # pipeline only break even, costing ~2.5us stalls per chunk).
        CH = 512

        def chunk_pieces(n_lc):
            # first piece 256 cols (1 l-chunk) for the shortest possible
            # denominator->first-av lead-in, then 512-col pieces
            cols = n_lc * Q
            pieces, off = [], 0
            while off < cols:
                w = min(Q if off == 0 else CH, cols - off)
                pieces.append((off, w))
                off += w
            return pieces

        r32_tiles = {}  # (ar_i, off) -> r32 tile with the cast denominator

        ps4_ctx = ExitStack()
        ps4 = ps4_ctx.enter_context(
            tc.tile_pool(name="ps4", bufs=4, space="PSUM"))
        for sl in range(LC // SLAB):
            kT_t = kT_pre[sl]
            for s in range(SLAB):
                lc = sl * SLAB + s
                ps_s = ps4.tile([P, Q], F32, tag="ps")
                for dc in range(DC):
                    nc.tensor.matmul(
                        ps_s,
                        kT_t[:, dc, s * P:(s + 1) * P],
                        qkT_sb[:, dc, :],
                        start=(dc == 0), stop=False,
                    )
                nc.tensor.matmul(
                    ps_s, ones_row[:, :P], qkb_sb,
                    start=False, stop=True,
                )
                nc.scalar.activation(
                    out=E_sb[:, lc * Q:(lc + 1) * Q], in_=ps_s,
                    func=mybir.ActivationFunctionType.Exp, scale=SCALE,
                )
                # chunk boundary: store this AR chunk's E and trigger its
                # AllReduce immediately (the CC queue drains chunks
                # back-to-back once the ~80us ncfw wall passes).
                if lc + 1 in ar_ends:
                    ar_i = ar_ends.index(lc + 1)
                    g0 = ar_starts[ar_i] * Q
                    W = AR_LCS[ar_i] * Q
                    nc.sync.dma_start(
                        out=E_drams[ar_i],
                        in_=E_sb[:, g0:g0 + W],
                    )
                    nc.gpsimd.collective_compute(
                        "AllReduce", mybir.AluOpType.add,
                        replica_groups=[list(range(N_CORES))],
                        ins=[E_drams[ar_i].opt()],
                        outs=[denom_drams[ar_i].opt()],
                    )
                    # dn piece loads + fp16->f32 casts for THIS chunk, on
                    # the Scalar queue (idle once phase C's exps retire; the
                    # triggers just wait there for the mesh's output).
                    for off, w in chunk_pieces(AR_LCS[ar_i]):
                        dn = rscr.tile([P, CH], ART, tag="dn")
                        nc.scalar.dma_start(
                            out=dn[:, :w],
                            in_=denom_drams[ar_i][:, off:off + w])
                        r32 = rscr.tile([P, CH], F32, tag="r32")
                        nc.scalar.copy(r32[:, :w], dn[:, :w])
                        r32_tiles[(ar_i, off)] = r32
                    if ar_i == len(AR_LCS) - 1:
                        # v + Wv prefetch (~7MB), deferred until the LAST
                        # chunk's E store (~62us) so the firmware-warmup
                        # window stays quiet; lands well before av (~115us).
                        for vq in range(G8 // 5):
                            nc.sync.dma_start(
                                out=v8_sb[:, vq * 5:(vq + 1) * 5, :, :],
                                in_=v8_in[:, vq * 5:(vq + 1) * 5, :, :])
                        for vq in range(3):
                            nc.sync.dma_start(
                                out=v_hi[:, vq * 4:(vq + 1) * 4, :],
                                in_=v_pv[:, vq * 4:(vq + 1) * 4, :])
                        nc.sync.dma_start(out=wvT_sb, in_=wvT_v)
        ps4_ctx.close()

        # ====== phases E+F interleaved per AR chunk: attn then av_T ======
        # attn = E * recip(denom). As soon as one AR chunk's denominator
        # lands (cast to f32 by the Scalar queue above), Vector runs
        # recip+mult per piece and the av_T matmuls follow -- overlapping
        # the remaining AllReduce chunk.
        with tc.tile_pool(name="accump", bufs=1, space="PSUM") as accump:
            # dc=7 shares its 2KB bank with the rowsum accumulator: PSUM
            # start=True zero-fills the WHOLE bank, so av_ps[7]'s chain
            # start (lc==0) also zeroes the rs region; the final rs matmul
            # then accumulates with start=False onto it. This keeps the rs
            # matmul free of the WAR dependency on the av-bank copies that
            # a separate rsp pool would have (saved ~2us of PE idle).
            av_ps = [accump.tile([P, Q], F32, name=f"av_ps_{dc}")
                     for dc in range(DC - 1)]
            av7rs = accump.tile([P, 2 * Q], F32, name="av_ps_7_rs")
            av_ps.append(av7rs[:, 0:Q])
            rs_ps = av7rs[:1, Q:Q + Q]
            DR = mybir.MatmulPerfMode.DoubleRow
            for ar_i, n_lc in enumerate(AR_LCS):
                first = ar_starts[ar_i] == 0
                for off, w in chunk_pieces(n_lc):
                    g = ar_starts[ar_i] * Q + off
                    sli = slice(g, g + w)
                    r32 = r32_tiles[(ar_i, off)]
                    nc.vector.reciprocal_approx_fast(r32[:, :w], r32[:, :w])
                    # chunk 0's attn quantizes to fp8 for the DoubleRow av
                    # (the E numerator itself stays fp16; only this chunk's
                    # attn weights carry fp8 rounding -> rel err ~1.6e-2)
                    dst = attn8_flat[:, sli] if first else \
                        attn_flat[:, g - FP8_LCS * Q:g - FP8_LCS * Q + w]
                    nc.vector.tensor_tensor(dst, E_sb[:, sli],
                                            r32[:, :w],
                                            op=mybir.AluOpType.mult)
                if first:
                    # fp8 DoubleRow av: one matmul per l-chunk PAIR per dc
                    # (2 k-tiles per instruction, ~1.44x over bf16)
                    for g2 in range(ar_ends[ar_i] // 2):
                        at8 = attn8_sb[:, g2, :, :]
                        for dc in range(DC):
                            nc.tensor.matmul(
                                av_ps[dc],
                                v8_sb[:, g2, :, dc * P:(dc + 1) * P], at8,
                                start=(g2 == 0), stop=False,
                                perf_mode=DR, skip_group_check=True,
                            )
                        for j in range(2):
                            lc = 2 * g2 + j
                            a1 = attn8_sb[:, g2, j, :]
                            if lc == 0:
                                nc.vector.tensor_copy(rs_acc, a1)
                            else:
                                nc.vector.tensor_tensor(
                                    rs_acc, rs_acc, a1,
                                    op=mybir.AluOpType.add)
                else:
                    for lc in range(ar_starts[ar_i], ar_ends[ar_i]):
                        lt = lc - FP8_LCS
                        at = attn_sb[:, lt, :]
                        for dc in range(DC):
                            nc.tensor.matmul(
                                av_ps[dc],
                                v_hi[:, lt, dc * P:(dc + 1) * P], at,
                                start=False, stop=(lc == LC - 1),
                                skip_group_check=True,
                            )
                        nc.vector.tensor_tensor(rs_acc, rs_acc, at,
                                                op=mybir.AluOpType.add)
            # collapse rowsum partials across partitions with one matmul.
            # start=False: the region was zeroed by av_ps[7]'s chain start.
            nc.tensor.matmul(rs_ps, ones_col, rs_acc,
                             start=False, stop=True)
            nc.vector.tensor_copy(rs_sb, rs_ps)
            # split the PSUM->SBUF copies across two engines AND by q-half
            # (out-proj iterates qm outer, so its qm=0 chains start after
            # only the first 8 half-copies)
            for qm in range(Q // P):
                for dc in range(DC):
                    dst = avT_sb[:, dc, qm * P:(qm + 1) * P]
                    src = av_ps[dc][:, qm * P:(qm + 1) * P]
                    if dc % 2 == 0:
                        nc.vector.tensor_copy(dst, src)
                    else:
                        nc.scalar.copy(dst, src)

        # ================ phase G: out projection ===============
        with (tc.tile_pool(name="outp", bufs=2, space="PSUM") as outp,
              tc.tile_pool(name="outsb", bufs=2) as outsb):
            # out[q,e] = sum_d av_T[d, q-chunk].T @ WvT[d, e] + rs * bv
            # The two eb chains are INTERLEAVED per dc: consecutive matmuls
            # then alternate PSUM banks, so each matmul's accumulator drain
            # and the next LDWEIGHTS overlap the sibling chain's matmul
            # (a single chain serializes at 0.37us/matmul vs 0.28 here),
            # and both chains share the same stationary avT chunk per dc.
            NB = D // 512
            for qm in range(Q // P):
                ps_o = [outp.tile([P, 512], F32, tag=f"ps_out{eb}",
                                  name=f"ps_o_{qm}_{eb}")
                        for eb in range(NB)]
                for dc in range(DC):
                    for eb in range(NB):
                        nc.tensor.matmul(
                            ps_o[eb],
                            avT_sb[:, dc, qm * P:(qm + 1) * P],
                            wvT_sb[:, dc, eb * 512:(eb + 1) * 512],
                            start=(dc == 0), stop=False,
                        )
                for eb in range(NB):
                    nc.tensor.matmul(
                        ps_o[eb],
                        rs_sb[:, qm * P:(qm + 1) * P],
                        bv_sb[:, eb * 512:(eb + 1) * 512],
                        start=False, stop=True,
                    )
                    o_sb = outsb.tile([P, 512], CDT, tag="o_sb")
                    nc.vector.tensor_copy(o_sb, ps_o[eb])
                    nc.sync.dma_start(
                        out=out_v[:, qm, eb * 512:(eb + 1) * 512], in_=o_sb)
        bigctx.close()
        kslab_ctx.close()

    nc.compile()
    return nc


def _prep_inputs(q, k, v, Wq, bq, Wk, bk, Wv, bv):
    """Shard + pre-transpose + cast on host. Returns in_maps for 8 cores."""
    cnp = _np_cdt()
    f32 = np.float32

    def c(x):
        return np.ascontiguousarray(np.asarray(x, dtype=f32), dtype=cnp)

    # shared across cores
    wqT = c(np.asarray(Wq, dtype=f32).T)
    wk_ = c(Wk)
    wvT = c(np.asarray(Wv, dtype=f32).T)
    bq_ = c(np.asarray(bq, dtype=f32).reshape(1, D))
    bk_ = c(np.asarray(bk, dtype=f32).reshape(EC, P).T)
    bv_ = c(np.asarray(bv, dtype=f32).reshape(1, D))
    ones_r = np.ones((1, Q), dtype=cnp)
    ones_c = np.ones((P, 1), dtype=cnp)

    import ml_dtypes

    FP8_LCS = AR_LCS[0]
    G8 = FP8_LCS // 2
    f8 = np.dtype(ml_dtypes.float8_e4m3)

    in_maps = []
    for b in range(B):
        # v8[p, g, j, d] = v[b][(2g+j)*128 + p, d] in fp8 (DoubleRow pairs)
        v_lo = np.asarray(v[b][:FP8_LCS * P], dtype=f32)
        v8 = np.ascontiguousarray(
            v_lo.reshape(G8, 2, P, D).transpose(2, 0, 1, 3), dtype=f8)
        in_maps.append({
            "kT": c(np.asarray(k[b], dtype=f32).T),
            "v8_in": v8,
            "v_in": c(v[b][FP8_LCS * P:]),
            "qT": c(np.asarray(q[b, :Q], dtype=f32).T),
            "wqT": wqT,
            "wk": wk_,
            "wvT": wvT,
            "bq_in": bq_,
            "bk_in": bk_,
            "bv_in": bv_,
            "ones_r": ones_r,
            "ones_c": ones_c,
        })
    return in_maps


def kernel(q, k, v, Wq, bq, Wk, bk, Wv, bv, _trace=False):
    q = np.asarray(q)
    k = np.asarray(k)
    v = np.asarray(v)
    if "nc" not in _cached:
        _cached["nc"] = build_kernel()
    nc = _cached["nc"]
    in_maps = _prep_inputs(q, k, v, Wq, bq, Wk, bk, Wv, bv)
    res = bass_utils.run_bass_kernel_spmd(
        nc, in_maps, core_ids=list(range(N_CORES)), trace=_trace)
    out = np.stack([res.results[c]["out"] for c in range(N_CORES)], axis=0)
    if _trace:
        _cached["last_results"] = res
    return out.astype(np.float32)


if __name__ == "__main__":
    rng = np.random.default_rng(0)
    ins = {
        "q": rng.standard_normal((B, L, D)).astype(np.float32),
        "k": rng.standard_normal((B, L, D)).astype(np.float32),
        "v": rng.standard_normal((B, L, D)).astype(np.float32),
        "Wq": (rng.standard_normal((D, D)) * 0.02).astype(np.float32),
        "bq": (rng.standard_normal(D) * 0.02).astype(np.float32),
        "bk": (rng.standard_normal(D) * 0.02).astype(np.float32),
        "Wk": (rng.standard_normal((D, D)) * 0.02).astype(np.float32),
        "Wv": (rng.standard_normal((D, D)) * 0.02).astype(np.float32),
        "bv": (rng.standard_normal(D) * 0.02).astype(np.float32),
    }
    out = kernel(**ins)
    print("out", out.shape, out.dtype)
